# revision 12
# baseline (speedup 1.0000x reference)
"""Trainium2 Bass kernel: batch-parallel LIF (leaky integrate-and-fire) scan.

Problem: x[128, 32, 8192] f32 -> spikes s[128, 32, 8192] f32 in {0, 1}.
    u' = u/2 + x_t ; s_t = (u' >= 1) ; u = u' * (1 - s_t)        (T=32 scan)

Sharding: data-parallel over batch (axis 0) across 8 NeuronCores, 16 rows each.
Per-core layout: timestep slab [16, 8192] -> SBUF [128 partitions, 1024 free]
(partition p = b*8 + k, free f, column n = k*1024 + f). The slab's columns are
split into independent lanes across DVE and GpSimd; each lane runs its own
serial scan, so engines never wait on each other inside a step.

mode "act_sign" (default): per lane-step two fused scalar_tensor_tensor ops
on the lane engine, compare offloaded to the otherwise-idle ACT engine:
  integrate: u' = (u mult 0.5) add x_t      STT on DVE / GpSimd
  compare:   g  = Sign(u' - 1) -> int8      ACT; spike iff g >= 0
  reset:     u  = (g is_lt 0) mult u'       STT on DVE / GpSimd
Host: s = (g >= 0).  (Sign only needs to be sign-correct around 0; exact ties
u' == 1 give g == 0 which both spike and reset correctly.)

mode "self_cmp" (fallback, bit-exact compare on the lane engines): rescale
time by 2^t (exact in fp32) so the decay multiply leaves the serial chain:
  prescale:  x~_t = 2^t * x_t               ACT, in place
  integrate: z += x~_t                      TT add
  compare:   c = (z < 2^t) -> int8          TS with immediate; spike iff c == 0
  reset:     z *= c                         TT mult
Host: s = (c == 0).
"""

import numpy as np

import concourse.bass as bass
import concourse.tile as tile
from concourse import bacc, mybir
from concourse.bass_utils import run_bass_kernel_spmd

B, T, N = 128, 32, 8192
NCORES = 8
BS = B // NCORES  # 16 batch rows per core
FW = 1024         # slab free width (8192 = 8 k-blocks * 1024)

CFG = dict(
    mode="hybrid",
    dve_cols=736,   # columns (out of 1024) on DVE; rest on GpSimd
    dve_sub=2,      # independent sub-lanes per engine
    gps_sub=2,
    xbufs=8,
    sbufs=6,
)

_cache = {}


def _build(cfg):
    V = cfg["dve_cols"]
    # hybrid: DVE lanes use fused STT + ACT Sign compare (STT is not
    # implemented for Pool); GpSimd lanes use the 2^t-rescaled form whose
    # three ops (TT add / TS compare / TT mult) are all Pool-legal.
    hybrid = cfg["mode"] == "hybrid"

    nc = bacc.Bacc("TRN2", target_bir_lowering=False, debug=False,
                   num_devices=NCORES)
    x = nc.dram_tensor("x", [BS, T, N], mybir.dt.float32,
                       kind="ExternalInput").ap()
    s = nc.dram_tensor("s", [BS, T, N], mybir.dt.int8,
                       kind="ExternalOutput").ap()
    # DMA AP limit is 3 dims: per-timestep transfers [b:16, k:8, f:1024].
    xr = x.rearrange("b t (k f) -> t b k f", f=FW)
    sr = s.rearrange("b t (k f) -> t b k f", f=FW)

    # lanes: (engine attr, f0, width)
    lanes = []

    def split(f0, f1, nsub, eng):
        w = (f1 - f0) // nsub
        offs = [f0 + j * w for j in range(nsub)] + [f1]
        for j in range(nsub):
            lanes.append((eng, offs[j], offs[j + 1] - offs[j]))

    split(0, V, cfg["dve_sub"], "vector")
    split(V, FW, cfg["gps_sub"], "gpsimd")

    with tile.TileContext(nc) as tc:
        with (
            tc.tile_pool(name="xp", bufs=cfg["xbufs"]) as xpool,
            tc.tile_pool(name="sp", bufs=cfg["sbufs"]) as spool,
            tc.tile_pool(name="up", bufs=1) as upool,
        ):
            bias = upool.tile([128, 1], mybir.dt.float32, tag="bias")
            nc.vector.memset(bias[:], -1.0)
            us = []
            for li, (eng, f0, w) in enumerate(lanes):
                u = upool.tile([128, w], mybir.dt.float32, tag=f"u{li}",
                               name=f"u{li}")
                getattr(nc, eng).memset(u[:], 0.0)
                us.append(u)

            for t in range(T):
                th = float(2 ** t)
                xt = xpool.tile([128, FW], mybir.dt.float32, tag="x",
                                name="xt")
                nc.sync.dma_start(out=xt[:], in_=xr[t])
                if hybrid:
                    # prescale only the GpSimd columns: x~_t = 2^t * x_t
                    nc.scalar.activation(xt[:, V:FW], xt[:, V:FW],
                                         mybir.ActivationFunctionType.Copy,
                                         bias=0.0, scale=th)
                st = spool.tile([128, FW], mybir.dt.int8, tag="s", name="st")
                for li, (eng, f0, w) in enumerate(lanes):
                    e = getattr(nc, eng)
                    z = us[li][:]
                    if eng == "vector":
                        e.scalar_tensor_tensor(
                            out=z, in0=z, scalar=0.5, in1=xt[:, f0:f0 + w],
                            op0=mybir.AluOpType.mult, op1=mybir.AluOpType.add)
                    else:
                        e.tensor_tensor(out=z, in0=z, in1=xt[:, f0:f0 + w],
                                        op=mybir.AluOpType.add)
                for li, (eng, f0, w) in enumerate(lanes):
                    e = getattr(nc, eng)
                    z = us[li][:]
                    ssl = st[:, f0:f0 + w]
                    if eng == "vector":
                        nc.scalar.activation(
                            ssl, z, mybir.ActivationFunctionType.Sign,
                            bias=bias[:], scale=1.0)
                    else:
                        e.tensor_single_scalar(out=ssl, in_=z, scalar=th,
                                               op=mybir.AluOpType.is_lt)
                for li, (eng, f0, w) in enumerate(lanes):
                    e = getattr(nc, eng)
                    z = us[li][:]
                    ssl = st[:, f0:f0 + w]
                    if eng == "vector":
                        e.scalar_tensor_tensor(
                            out=z, in0=ssl, scalar=0.0, in1=z,
                            op0=mybir.AluOpType.is_lt,
                            op1=mybir.AluOpType.mult)
                    else:
                        e.tensor_tensor(out=z, in0=z, in1=ssl,
                                        op=mybir.AluOpType.mult)
                # stores issue from the ACT HWDGE queue so they never
                # head-of-line-block the loads on the SP queue
                nc.scalar.dma_start(out=sr[t], in_=st[:])
    nc.compile()
    return nc


def _get_nc(cfg=None):
    cfg = dict(CFG if cfg is None else cfg)
    key = tuple(sorted(cfg.items()))
    if key not in _cache:
        _cache[key] = (_build(cfg), cfg)
    return _cache[key]


def _postprocess(g: np.ndarray, cfg) -> np.ndarray:
    # DVE columns (f < V within each 1024 block) hold g = sign(u'-1):
    # spike iff g >= 0. GpSimd columns hold c = (z < 2^t): spike iff c == 0.
    V = cfg["dve_cols"]
    gk = g.reshape(-1, FW)
    s = np.empty(gk.shape, dtype=np.float32)
    s[:, :V] = gk[:, :V] >= 0
    s[:, V:] = gk[:, V:] == 0
    return s.reshape(g.shape)


def run(x: np.ndarray, cfg=None, trace: bool = False):
    """Run on 8 cores; returns (spikes f32 [128,32,8192], BassKernelResults)."""
    nc, cfg = _get_nc(cfg)
    in_maps = [{"x": np.ascontiguousarray(x[c * BS:(c + 1) * BS])}
               for c in range(NCORES)]
    res = run_bass_kernel_spmd(nc, in_maps, list(range(NCORES)), trace=trace)
    g = np.concatenate([res.results[c]["s"] for c in range(NCORES)], axis=0)
    return _postprocess(g, cfg), res


def kernel(x: np.ndarray) -> np.ndarray:
    out, _ = run(np.asarray(x))
    return out
